# revision 1
# baseline (speedup 1.0000x reference)
"""AVWGCN (adaptive-embedding graph conv) Trainium2 Bass kernel.

Math (reference):
    A   = softmax(relu(E E^T), axis=1)            # [N, N], E: [N, D]
    T0  = I, T1 = A, T2 = 2 A A - I               # Chebyshev supports
    W   = einsum('nd,dkio->nkio', E, Wp)          # per-node weights
    b   = E @ bp                                  # per-node bias
    x_g = einsum('knm,bmc->bnkc', T, x)
    out = einsum('bnki,nkio->bno', x_g, W) + b

Restructuring used here (all algebraically exact, fp32):
  * Z := exp(relu(E E^T)) = max(exp(E E^T), 1) is SYMMETRIC; with row sums
    s, A = Z/s.  All aggregation matmuls use Z tiles as lhsT directly
    (lhsT.T @ rhs with symmetric Z) and fold 1/s into output scaling.
  * y1 = A @ X,  u2' = 2 A y1  (so y2 = u2' - X^T);  the "- X^T" is folded
    into the weights: W_eff[k0] = Wp[k0] - Wp[k2] applied to x, Wp[k2]
    applied to u2'.
  * Final per-node contraction is d-expanded:
    H[n,b,(d,o)] = sum_ki R[ki,(b,n)] Wm[ki,(d,o)],  out = sum_d E[n,d] H
    where R = [x^T; y1^T; u2'^T] ([k*C rows, (b,n) cols]), Wm = Wp_eff.
    The bias is folded in as an extra all-ones contraction row whose weight
    row is bp flattened over (d,o).

Sharding: data-parallel over batch B: 8 cores x 8 batches, zero comm.
"""

import os
import sys
import threading

sys.path.insert(0, "/opt/trn_rl_repo")

import numpy as np

import concourse.bass as bass  # noqa: E402
import concourse.mybir as mybir  # noqa: E402
from concourse import bacc  # noqa: E402
from concourse.tile import TileContext  # noqa: E402
from concourse.masks import make_identity  # noqa: E402
from concourse.bass_utils import run_bass_kernel_spmd  # noqa: E402

F32 = mybir.dt.float32
F32R = mybir.dt.float32r
F16 = mybir.dt.float16
AF = mybir.ActivationFunctionType
OP = mybir.AluOpType

NCORES = 8
B = 64
BSH = B // NCORES          # 8 batches per core
N = 2048
C = 64                     # C_IN == C_OUT
D = 16                     # embedding dim
K = 3                      # Chebyshev order
P = 128                    # partitions
NT = N // P                # 16 node blocks
BC = BSH * C               # 512 = per-core (b, c) width
KI = K * C                 # 192 contraction for the final stage

USE_F32R = os.environ.get("KERNEL_F32", "") != "1"
MF = F32R if USE_F32R else F32   # dtype for all PE-matmul operands
ACT_D_SPLIT = 8            # d < split scaled on ACT, rest on DVE


def _rc(ap):
    """Reinterpret an fp32 DRAM AP as float32r for DMA dtype matching."""
    return ap.bitcast(F32R) if USE_F32R else ap


def build_program():
    nc = bacc.Bacc("TRN2", target_bir_lowering=False, debug=False,
                   num_devices=NCORES)

    x_in = nc.dram_tensor("x", [BSH, N, C], F32, kind="ExternalInput")
    emb = nc.dram_tensor("emb", [N, D], F32, kind="ExternalInput")
    wp = nc.dram_tensor("wp", [D, K, C, C], F32, kind="ExternalInput")
    bp = nc.dram_tensor("bp", [D, C], F32, kind="ExternalInput")
    out_d = nc.dram_tensor("out", [BSH, N, C], F32, kind="ExternalOutput")
    z_dram = nc.dram_tensor("zd", [N, N], MF)  # internal bounce for Z

    with TileContext(nc) as tc:
        with tc.tile_pool(name="persist", bufs=1) as pp:
            ident = pp.tile([P, P], F32)
            make_identity(nc, ident[:])

            # E node-major: [128, (t, d)] and transposed ET [16, 2048]
            e_sb = pp.tile([P, NT * D], F32)
            for t in range(NT):
                nc.sync.dma_start(out=e_sb[:, t * D:(t + 1) * D],
                                  in_=emb[t * P:(t + 1) * P, :])
            et = pp.tile([D, N], MF)

            # Final-stage weights, (d, o)-major columns.
            wm0 = pp.tile([P, D * C], MF)      # rows (k0 c | k1 c)
            wm1 = pp.tile([KI - P + 1, D * C], MF)  # rows k2 c + ones-row
            for k in range(2):
                nc.sync.dma_start(
                    out=wm0[k * C:(k + 1) * C, :].rearrange(
                        "c (d o) -> c d o", o=C),
                    in_=_rc(wp[:, k, :, :].transpose([1, 0, 2])))
            nc.sync.dma_start(
                out=wm1[0:C, :].rearrange("c (d o) -> c d o", o=C),
                in_=_rc(wp[:, 2, :, :].transpose([1, 0, 2])))
            # bias row: H[n, (d, o)] += 1 * bp[d, o]
            nc.sync.dma_start(
                out=wm1[C:C + 1, :].rearrange("a (d o) -> a d o", o=C),
                in_=_rc(bp[:, :].unsqueeze(0)))
            # W_eff[k0] = Wp[k0] - Wp[k2]
            nc.vector.tensor_tensor(out=wm0[0:C, :], in0=wm0[0:C, :],
                                    in1=wm1[0:C, :], op=OP.subtract)

            s_all = pp.tile([P, 2 * NT], F32)
            s_sb = s_all[:, 0:NT]
            sinv = s_all[:, NT:2 * NT]
            sinv2 = pp.tile([P, NT], F32)
            rc0 = pp.tile([P, BSH * N], MF)   # rows: x^T (c) | y1^T (c)

            # ---- build ET via PE transpose ----
            with tc.tile_pool(name="pet", bufs=4, space="PSUM") as pet:
                for t in range(NT):
                    ptile = pet.tile([D, P], F32)
                    nc.tensor.transpose(ptile[:], e_sb[:, t * D:(t + 1) * D],
                                        ident[:])
                    nc.any.tensor_copy(et[:, t * P:(t + 1) * P], ptile[:])

            # ================= Phase B: Z = max(exp(E E^T), 1), s ========
            with tc.tile_pool(name="zb", bufs=2) as zbp, \
                 tc.tile_pool(name="psz", bufs=8, space="PSUM") as psz:
                for t in range(NT):
                    zbt = zbp.tile([P, N], MF)
                    for j in range(4):
                        zt = psz.tile([P, 512], F32)
                        nc.tensor.matmul(zt[:], et[:, t * P:(t + 1) * P],
                                         et[:, j * 512:(j + 1) * 512],
                                         start=True, stop=True)
                        nc.scalar.activation(zbt[:, j * 512:(j + 1) * 512],
                                             zt[:], AF.Exp)
                    # z = max(z, 1);  s[row] = sum(z)
                    nc.vector.tensor_scalar_max(zbt[:], zbt[:], 1.0)
                    nc.vector.tensor_reduce(
                        out=s_sb[:, t:t + 1], in_=zbt[:],
                        axis=mybir.AxisListType.X, op=OP.add)
                    nc.sync.dma_start(out=z_dram[t * P:(t + 1) * P, :],
                                      in_=zbt[:])

            nc.vector.reciprocal(sinv, s_sb)
            nc.vector.tensor_scalar_mul(sinv2[:], sinv, 2.0)

            with tc.tile_pool(name="poolA", bufs=1) as pa:
                y1 = pa.tile([P, NT * BC], MF)
                srep = pa.tile([P, N], F32)   # 2/s[n] replicated on all rows
                srow = pa.tile([1, N], F32)
                with tc.tile_pool(name="pst", bufs=1) as pst, \
                     tc.tile_pool(name="psts", bufs=1, space="PSUM") as psts:
                    stp = psts.tile([D, P], F32)
                    nc.tensor.transpose(stp[:], sinv2[:], ident[:])
                    st_sb = pst.tile([D, P], F32)
                    nc.any.tensor_copy(st_sb[:], stp[:])
                    for t in range(D):
                        nc.sync.dma_start(
                            out=srow[0:1, t * P:(t + 1) * P],
                            in_=st_sb[t:t + 1, :])
                nc.gpsimd.partition_broadcast(srep[:], srow[0:1, :])

                # ============= Phase C: y1 = (Z @ X) / s and x^T =========
                with tc.tile_pool(name="poolB", bufs=1) as pb, \
                     tc.tile_pool(name="zc", bufs=2) as zcp, \
                     tc.tile_pool(name="pagg", bufs=2, space="PSUM") as pagg, \
                     tc.tile_pool(name="ptr", bufs=6, space="PSUM") as ptr:
                    xs = pb.tile([P, NT * BC], MF)
                    for m in range(NT):
                        nc.sync.dma_start(
                            out=xs[:, m * BC:(m + 1) * BC].rearrange(
                                "p (b c) -> p b c", c=C),
                            in_=_rc(x_in[:, m * P:(m + 1) * P, :].transpose(
                                [1, 0, 2])))
                    for t in range(NT):
                        zc = zcp.tile([P, N], MF)
                        nc.sync.dma_start(
                            out=zc[:].rearrange("p (mb c) -> p mb c", c=P),
                            in_=z_dram[:, t * P:(t + 1) * P].rearrange(
                                "(mb p) c -> p mb c", p=P))
                        u1 = pagg.tile([P, BC], F32)
                        for m in range(NT):
                            nc.tensor.matmul(
                                u1[:], zc[:, m * P:(m + 1) * P],
                                xs[:, m * BC:(m + 1) * BC],
                                start=(m == 0), stop=(m == NT - 1))
                        nc.vector.tensor_scalar_mul(
                            y1[:, t * BC:(t + 1) * BC], u1[:],
                            sinv[:, t:t + 1])
                    # x^T into Rc0 rows 0..C (k0 slot)
                    for m in range(NT):
                        for b in range(BSH):
                            pt = ptr.tile([P, P], F32)
                            nc.tensor.transpose(
                                pt[0:C, :],
                                xs[:, m * BC + b * C:
                                   m * BC + (b + 1) * C].bitcast(F32),
                                ident[:])
                            nc.any.tensor_copy(
                                rc0[0:C, b * N + m * P:b * N + (m + 1) * P],
                                pt[0:C, :])

                # ============= Phase D: y1^T, u2' = 2 (Z @ y1) / s =======
                # Transpose-mode PSUM outputs must start at partition 0, so
                # stage [64, b*128] per m-block and DMA-shift into rows 64..128.
                with tc.tile_pool(name="ptr2", bufs=8, space="PSUM") as ptr2, \
                     tc.tile_pool(name="stg", bufs=2) as stgp:
                    for m in range(NT):
                        stg = stgp.tile([C, BSH * P], MF)
                        for b in range(BSH):
                            pt = ptr2.tile([P, P], F32)
                            nc.tensor.transpose(
                                pt[0:C, :],
                                y1[:, m * BC + b * C:
                                   m * BC + (b + 1) * C].bitcast(F32),
                                ident[:])
                            nc.any.tensor_copy(
                                stg[:, b * P:(b + 1) * P], pt[0:C, :])
                        nc.sync.dma_start(
                            out=rc0[C:P, :].rearrange(
                                "q (b n) -> q b n", b=BSH)[:, :,
                                                          m * P:(m + 1) * P],
                            in_=stg[:].rearrange("q (b n) -> q b n", b=BSH))

                with tc.tile_pool(name="poolC", bufs=1) as pc:
                    rc1 = pc.tile([KI - P + 1, BSH * N], MF)
                    # bias ones-row (memset cannot write f32r; cast-copy can)
                    ones_f32 = pc.tile([1, 512], F32)
                    nc.vector.memset(ones_f32[:], 1.0)
                    for q in range(BSH * N // 512):
                        nc.any.tensor_copy(
                            rc1[C:C + 1, q * 512:(q + 1) * 512], ones_f32[:])
                    with tc.tile_pool(name="zl", bufs=3) as zlp, \
                         tc.tile_pool(name="pu2", bufs=2,
                                      space="PSUM") as pu2, \
                         tc.tile_pool(name="y2p", bufs=2) as y2p:
                        for nq in range(4):
                            u2t = pu2.tile([P, 4 * 512], F32)
                            for m in range(NT):
                                zl = zlp.tile([P, 512], MF)
                                nc.sync.dma_start(
                                    out=zl[:],
                                    in_=z_dram[m * P:(m + 1) * P,
                                               nq * 512:(nq + 1) * 512])
                                for bc in range(4):
                                    nc.tensor.matmul(
                                        u2t[:, bc * 512:(bc + 1) * 512],
                                        y1[:, m * BC + bc * P:
                                           m * BC + (bc + 1) * P],
                                        zl[:],
                                        start=(m == 0), stop=(m == NT - 1))
                            for bc in range(4):
                                y2t = y2p.tile([P, 512], MF)
                                nc.vector.tensor_tensor(
                                    out=y2t[:],
                                    in0=u2t[:, bc * 512:(bc + 1) * 512],
                                    in1=srep[:, nq * 512:(nq + 1) * 512],
                                    op=OP.mult)
                                for h in range(2):
                                    bb = 2 * bc + h
                                    nc.sync.dma_start(
                                        out=rc1[0:C, bb * N + nq * 512:
                                                bb * N + (nq + 1) * 512],
                                        in_=y2t[h * C:(h + 1) * C, :])

                    # ============= Phase E: final ========================
                    # (poolA freed only at its context exit; keep E inside
                    #  poolC scope so rc1 stays live.)
                    with tc.tile_pool(name="psH", bufs=2, space="PSUM") as psh, \
                         tc.tile_pool(name="hp", bufs=2) as hpp, \
                         tc.tile_pool(name="outp", bufs=2) as outp:
                        for t in range(NT):
                            out_nt = outp.tile([P, BC], F32)
                            for bg in range(4):
                                hps = psh.tile([P, 2 * D * C], F32)
                                for br in range(2):
                                    b = bg * 2 + br
                                    cs = b * N + t * P
                                    for h in range(2):
                                        hsl = hps[:, br * 1024 + h * 512:
                                                  br * 1024 + (h + 1) * 512]
                                        nc.tensor.matmul(
                                            hsl, rc0[:, cs:cs + P],
                                            wm0[:, h * 512:(h + 1) * 512],
                                            start=True, stop=False)
                                        nc.tensor.matmul(
                                            hsl, rc1[:, cs:cs + P],
                                            wm1[:, h * 512:(h + 1) * 512],
                                            start=False, stop=True)
                                # scale by E[n, d] with (o, d) relayout
                                hpt = hpp.tile([P, 2 * D * C], F16)
                                for d in range(D):
                                    src = hps[:].rearrange(
                                        "p (br d o) -> p br d o", br=2, d=D
                                    )[:, :, d, :]
                                    dst = hpt[:].rearrange(
                                        "p (br o d) -> p br o d", br=2, d=D
                                    )[:, :, :, d]
                                    scal = e_sb[:, t * D + d:
                                                t * D + d + 1]
                                    if d < ACT_D_SPLIT:
                                        nc.scalar.activation(
                                            dst, src, AF.Copy, scale=scal)
                                    else:
                                        nc.vector.tensor_scalar_mul(
                                            dst, src, scal)
                                nc.vector.tensor_reduce(
                                    out=out_nt[:, bg * P:(bg + 1) * P],
                                    in_=hpt[:].rearrange(
                                        "p (br o d) -> p (br o) d", br=2, d=D),
                                    axis=mybir.AxisListType.X, op=OP.add)
                            nc.sync.dma_start(
                                out=out_d[:, t * P:(t + 1) * P, :].transpose(
                                    [1, 0, 2]),
                                in_=out_nt[:].rearrange(
                                    "p (b c) -> p b c", c=C))

    nc.compile()
    return nc


_CACHE = {}
_LOCK = threading.Lock()


def _get_program():
    with _LOCK:
        if "nc" not in _CACHE:
            _CACHE["nc"] = build_program()
        return _CACHE["nc"]


def kernel(x, node_embeddings, weights_pool, bias_pool):
    x = np.ascontiguousarray(np.asarray(x, dtype=np.float32))
    emb = np.ascontiguousarray(np.asarray(node_embeddings, dtype=np.float32))
    wp = np.ascontiguousarray(np.asarray(weights_pool, dtype=np.float32))
    bp = np.ascontiguousarray(np.asarray(bias_pool, dtype=np.float32))

    nc = _get_program()
    core_ids = list(range(NCORES))
    in_maps = [
        {"x": x[i * BSH:(i + 1) * BSH], "emb": emb, "wp": wp, "bp": bp}
        for i in core_ids
    ]
    trace = os.environ.get("KERNEL_TRACE", "") == "1"
    res = run_bass_kernel_spmd(nc, in_maps, core_ids, trace=trace)
    if trace:
        kernel.last_exec_time_ns = res.exec_time_ns
        kernel.last_results = res
    out = np.concatenate([res.results[i]["out"] for i in core_ids], axis=0)
    return out


kernel.last_exec_time_ns = None

if __name__ == "__main__":
    rng = np.random.default_rng(0)
    ins = {
        "x": rng.standard_normal((B, N, C), dtype=np.float32),
        "node_embeddings": rng.standard_normal((N, D), dtype=np.float32),
        "weights_pool": (rng.standard_normal((D, K, C, C), dtype=np.float32)
                         * 0.1),
        "bias_pool": rng.standard_normal((D, C), dtype=np.float32) * 0.1,
    }
    out = kernel(**ins)
    print("out", out.shape, out.dtype, float(np.abs(out).mean()))



# revision 5
# speedup vs baseline: 1.6105x; 1.6105x over previous
"""AVWGCN (adaptive-embedding graph conv) Trainium2 Bass kernel.

Math (reference):
    A   = softmax(relu(E E^T), axis=1)            # [N, N], E: [N, D]
    T0  = I, T1 = A, T2 = 2 A A - I               # Chebyshev supports
    W   = einsum('nd,dkio->nkio', E, Wp)          # per-node weights
    b   = E @ bp                                  # per-node bias
    x_g = einsum('knm,bmc->bnkc', T, x)
    out = einsum('bnki,nkio->bno', x_g, W) + b

Restructuring used here (all algebraically exact up to bf16 rounding):
  * Z := exp(relu(E E^T)) = max(exp(E E^T), 1) is SYMMETRIC; with row sums
    s, A = Z/s.  All aggregation matmuls use Z tiles as lhsT directly
    (lhsT.T @ rhs with symmetric Z) and fold 1/s into output scaling.
  * y1 = A @ X,  u2' = 2 A y1  (so y2 = u2' - X^T);  the "- X^T" is folded
    into the weights: W_eff[k0] = Wp[k0] - Wp[k2] applied to x, Wp[k2]
    applied to u2'.
  * Final per-node contraction is d-expanded with (o, d)-major columns:
    H[n,b,(o,d)] = sum_ki R[ki,(b,n)] Wm[ki,(o,d)],
    out[n,b,o] = sum_d E[n,d] H[n,b,(o,d)]
    where R = [x^T; y1^T; u2'^T] ([k*C rows, (b,n) cols]), Wm = Wp_eff.
    The bias is folded in as an extra all-ones contraction row whose weight
    row is bp flattened over (o,d).  The (o,d)-major layout makes the
    E-scale a single stride-0-broadcast multiply and the d-reduction a
    single minor-axis tensor_reduce per PSUM tile.

Sharding: data-parallel over batch B: 8 cores x 8 batches, zero comm.
All matmul operands are bf16 (PSUM accumulation stays fp32).
"""

import os
import sys
import threading

sys.path.insert(0, "/opt/trn_rl_repo")

import numpy as np

import concourse.bass as bass  # noqa: E402
import concourse.mybir as mybir  # noqa: E402
from concourse import bacc  # noqa: E402
from concourse.tile import TileContext  # noqa: E402
from concourse.masks import make_identity  # noqa: E402
from concourse.bass_utils import run_bass_kernel_spmd  # noqa: E402

F32 = mybir.dt.float32
BF = mybir.dt.bfloat16
F16 = mybir.dt.float16
AF = mybir.ActivationFunctionType
OP = mybir.AluOpType

NCORES = 8
B = 64
BSH = B // NCORES          # 8 batches per core
N = 2048
C = 64                     # C_IN == C_OUT
D = 16                     # embedding dim
K = 3                      # Chebyshev order
P = 128                    # partitions
NT = N // P                # 16 node blocks
BC = BSH * C               # 512 = per-core (b, c) width
KI = K * C                 # 192 contraction for the final stage
DC = D * C                 # 1024 (o, d) columns per b in phase E


def build_program():
    nc = bacc.Bacc("TRN2", target_bir_lowering=False, debug=False,
                   num_devices=NCORES)

    x_in = nc.dram_tensor("x", [BSH, N, C], F32, kind="ExternalInput")
    emb = nc.dram_tensor("emb", [N, D], F32, kind="ExternalInput")
    wp = nc.dram_tensor("wp", [D, K, C, C], F32, kind="ExternalInput")
    bp = nc.dram_tensor("bp", [D, C], F32, kind="ExternalInput")
    out_d = nc.dram_tensor("out", [BSH, N, C], F32, kind="ExternalOutput")
    z_dram = nc.dram_tensor("zd", [N, N], BF)  # internal bounce for Z

    with TileContext(nc) as tc:
        with tc.tile_pool(name="persist", bufs=1) as pp:
            ident = pp.tile([P, P], F32)
            make_identity(nc, ident[:])
            identb = pp.tile([P, P], BF)
            make_identity(nc, identb[:])

            # E node-major: [128, (t, d)] and transposed ET [16, 2048] bf16
            e_sb = pp.tile([P, NT * D], F32)
            for t in range(NT):
                nc.sync.dma_start(out=e_sb[:, t * D:(t + 1) * D],
                                  in_=emb[t * P:(t + 1) * P, :])
            et = pp.tile([D, N], BF)

            # Final-stage weights, (o, d)-major columns, bf16.
            wm0 = pp.tile([P, DC], BF)           # rows (k0 c | k1 c)
            wm1 = pp.tile([KI - P + 1, DC], BF)  # rows k2 c + ones-row
            with tc.tile_pool(name="wst", bufs=1) as wst:
                # staging loads keep the DRAM-side layout (d, o) contiguous
                wmst = wst.tile([C, K * DC], F32)
                for k in range(K):
                    nc.sync.dma_start(
                        out=wmst[:, k * DC:(k + 1) * DC].rearrange(
                            "c (d o) -> c d o", o=C),
                        in_=wp[:, k, :, :].transpose([1, 0, 2]))
                bpst = wst.tile([1, DC], F32)
                nc.sync.dma_start(
                    out=bpst[:].rearrange("a (d o) -> a d o", o=C),
                    in_=bp[:, :].unsqueeze(0))

                def odmaj(ap):
                    # view an (o, d)-major destination in (d, o) order
                    return ap.rearrange("c (o d) -> c d o", d=D)

                def domaj(ap):
                    return ap.rearrange("c (d o) -> c d o", o=C)

                # W_eff[k0] = Wp[k0] - Wp[k2]; cast + (o,d)-relayout on write
                nc.vector.tensor_tensor(
                    out=odmaj(wm0[0:C, :]),
                    in0=domaj(wmst[:, 0 * DC:1 * DC]),
                    in1=domaj(wmst[:, 2 * DC:3 * DC]), op=OP.subtract)
                nc.vector.tensor_copy(odmaj(wm0[C:2 * C, :]),
                                      domaj(wmst[:, 1 * DC:2 * DC]))
                nc.vector.tensor_copy(odmaj(wm1[0:C, :]),
                                      domaj(wmst[:, 2 * DC:3 * DC]))
                nc.vector.tensor_copy(
                    wm1[C:C + 1, :].rearrange("c (o d) -> c d o", d=D),
                    bpst[:].rearrange("a (d o) -> a d o", o=C))

            s_all = pp.tile([P, 2 * NT], F32)
            s_sb = s_all[:, 0:NT]
            sinv = s_all[:, NT:2 * NT]
            sinv2 = pp.tile([P, NT], F32)
            rc0 = pp.tile([P, BSH * N], BF)   # rows: x^T (c) | y1^T (c)

            # ---- build ET via PE transpose (f32 in, cast on copy) ----
            with tc.tile_pool(name="pet", bufs=4, space="PSUM") as pet:
                for t in range(NT):
                    ptile = pet.tile([D, P], F32)
                    nc.tensor.transpose(ptile[:], e_sb[:, t * D:(t + 1) * D],
                                        ident[:])
                    nc.any.tensor_copy(et[:, t * P:(t + 1) * P], ptile[:])

            # ================= Phase B: Z = max(exp(E E^T), 1), s ========
            with tc.tile_pool(name="zb", bufs=2) as zbp, \
                 tc.tile_pool(name="psz", bufs=8, space="PSUM") as psz:
                for t in range(NT):
                    zbt = zbp.tile([P, N], BF)
                    for j in range(4):
                        zt = psz.tile([P, 512], F32)
                        nc.tensor.matmul(zt[:], et[:, t * P:(t + 1) * P],
                                         et[:, j * 512:(j + 1) * 512],
                                         start=True, stop=True)
                        nc.scalar.activation(zbt[:, j * 512:(j + 1) * 512],
                                             zt[:], AF.Exp)
                    # z = max(z, 1);  s[row] = sum(z)
                    nc.vector.tensor_scalar_max(zbt[:], zbt[:], 1.0)
                    nc.vector.tensor_reduce(
                        out=s_sb[:, t:t + 1], in_=zbt[:],
                        axis=mybir.AxisListType.X, op=OP.add)
                    nc.sync.dma_start(out=z_dram[t * P:(t + 1) * P, :],
                                      in_=zbt[:])

            nc.vector.reciprocal(sinv, s_sb)
            nc.vector.tensor_scalar_mul(sinv2[:], sinv, 2.0)

            with tc.tile_pool(name="poolA", bufs=1) as pa:
                y1 = pa.tile([P, NT * BC], BF)
                srep = pa.tile([P, N], F32)   # 2/s[n] replicated on all rows
                srow = pa.tile([1, N], F32)
                with tc.tile_pool(name="pst", bufs=1) as pst, \
                     tc.tile_pool(name="psts", bufs=1, space="PSUM") as psts:
                    stp = psts.tile([D, P], F32)
                    nc.tensor.transpose(stp[:], sinv2[:], ident[:])
                    st_sb = pst.tile([D, P], F32)
                    nc.any.tensor_copy(st_sb[:], stp[:])
                    for t in range(D):
                        nc.sync.dma_start(
                            out=srow[0:1, t * P:(t + 1) * P],
                            in_=st_sb[t:t + 1, :])
                nc.gpsimd.partition_broadcast(srep[:], srow[0:1, :])

                # ============= Phase C: y1 = (Z @ X) / s and x^T =========
                with tc.tile_pool(name="poolB", bufs=1) as pb, \
                     tc.tile_pool(name="xst", bufs=2) as xstp, \
                     tc.tile_pool(name="zc", bufs=2) as zcp, \
                     tc.tile_pool(name="pagg", bufs=2, space="PSUM") as pagg, \
                     tc.tile_pool(name="ptx", bufs=2, space="PSUM") as ptxp:
                    xs = pb.tile([P, NT * BC], BF)
                    for m in range(NT):
                        xst = xstp.tile([P, BC], F32)
                        nc.sync.dma_start(
                            out=xst[:].rearrange("p (b c) -> p b c", c=C),
                            in_=x_in[:, m * P:(m + 1) * P, :].transpose(
                                [1, 0, 2]))
                        nc.scalar.activation(xs[:, m * BC:(m + 1) * BC],
                                             xst[:], AF.Copy)
                    for t in range(NT):
                        zc = zcp.tile([P, N], BF)
                        nc.sync.dma_start(
                            out=zc[:].rearrange("p (mb c) -> p mb c", c=P),
                            in_=z_dram[:, t * P:(t + 1) * P].rearrange(
                                "(mb p) c -> p mb c", p=P))
                        u1 = pagg.tile([P, BC], F32)
                        for m in range(NT):
                            nc.tensor.matmul(
                                u1[:], zc[:, m * P:(m + 1) * P],
                                xs[:, m * BC:(m + 1) * BC],
                                start=(m == 0), stop=(m == NT - 1))
                        nc.vector.tensor_scalar_mul(
                            y1[:, t * BC:(t + 1) * BC], u1[:],
                            sinv[:, t:t + 1])
                    # x^T into Rc0 rows 0..C (k0 slot): batch 8 transposes
                    # per m-block into one PSUM tile, one strided copy out.
                    for m in range(NT):
                        ptx = ptxp.tile([C, BSH * P], BF)
                        for b in range(BSH):
                            nc.tensor.transpose(
                                ptx[:, b * P:(b + 1) * P],
                                xs[:, m * BC + b * C:m * BC + (b + 1) * C],
                                identb[:])
                        nc.any.tensor_copy(
                            rc0[0:C, :].rearrange(
                                "c (b n) -> c b n", b=BSH)[:, :,
                                                           m * P:(m + 1) * P],
                            ptx[:].rearrange("c (b n) -> c b n", b=BSH))

                # ============= Phase D: y1^T, u2' = 2 (Z @ y1) / s =======
                # y1^T transposes target PSUM partitions 64..127 directly
                # (tile_position col offset 64), then a lane-locked copy
                # lands them in rc0 rows 64..127.
                with tc.tile_pool(name="pty", bufs=2, space="PSUM") as ptyp:
                    for m in range(NT):
                        pty = ptyp.tile([P, BSH * P], BF)
                        for b in range(BSH):
                            nc.tensor.transpose(
                                pty[C:P, b * P:(b + 1) * P],
                                y1[:, m * BC + b * C:m * BC + (b + 1) * C],
                                identb[:],
                                tile_position=(0, C))
                        nc.any.tensor_copy(
                            rc0[C:P, :].rearrange(
                                "c (b n) -> c b n", b=BSH)[:, :,
                                                           m * P:(m + 1) * P],
                            pty[C:P, :].rearrange("c (b n) -> c b n", b=BSH))

                with tc.tile_pool(name="poolC", bufs=1) as pcp:
                    rc1 = pcp.tile([KI - P + 1, BSH * N], BF)
                    nc.vector.memset(rc1[C:C + 1, :], 1.0)
                    with tc.tile_pool(name="zl", bufs=3) as zlp, \
                         tc.tile_pool(name="pu2", bufs=2,
                                      space="PSUM") as pu2, \
                         tc.tile_pool(name="y2p", bufs=2) as y2p:
                        for nq in range(4):
                            u2t = pu2.tile([P, 4 * 512], F32)
                            for m in range(NT):
                                zl = zlp.tile([P, 512], BF)
                                nc.sync.dma_start(
                                    out=zl[:],
                                    in_=z_dram[m * P:(m + 1) * P,
                                               nq * 512:(nq + 1) * 512])
                                for bc in range(4):
                                    nc.tensor.matmul(
                                        u2t[:, bc * 512:(bc + 1) * 512],
                                        y1[:, m * BC + bc * P:
                                           m * BC + (bc + 1) * P],
                                        zl[:],
                                        start=(m == 0), stop=(m == NT - 1))
                            # y2 = u2t * (2/s[n]) in one broadcast multiply
                            y2t = y2p.tile([P, 4 * 512], BF)
                            nc.vector.tensor_tensor(
                                out=y2t[:].rearrange(
                                    "p (q n) -> p q n", q=4),
                                in0=u2t[:].rearrange(
                                    "p (q n) -> p q n", q=4),
                                in1=srep[:, nq * 512:(nq + 1) * 512]
                                .unsqueeze(1).broadcast_to((P, 4, 512)),
                                op=OP.mult)
                            for bc in range(4):
                                for h in range(2):
                                    bb = 2 * bc + h
                                    dst = rc1[0:C, bb * N + nq * 512:
                                              bb * N + (nq + 1) * 512]
                                    src = y2t[h * C:(h + 1) * C,
                                              bc * 512:(bc + 1) * 512]
                                    if h == 0:
                                        nc.scalar.activation(dst, src,
                                                             AF.Copy)
                                    else:
                                        nc.sync.dma_start(out=dst, in_=src)

                    # ============= Phase E: final ========================
                    with tc.tile_pool(name="psH", bufs=2,
                                      space="PSUM") as psh, \
                         tc.tile_pool(name="hp", bufs=2) as hpp, \
                         tc.tile_pool(name="outp", bufs=2) as outp:
                        for t in range(NT):
                            out_nt = outp.tile([P, BC], F32)
                            for bg in range(4):
                                hps = psh.tile([P, 2 * DC], F32)
                                for br in range(2):
                                    b = bg * 2 + br
                                    cs = b * N + t * P
                                    for h in range(2):
                                        hsl = hps[:, br * DC + h * 512:
                                                  br * DC + (h + 1) * 512]
                                        nc.tensor.matmul(
                                            hsl, rc0[:, cs:cs + P],
                                            wm0[:, h * 512:(h + 1) * 512],
                                            start=True, stop=False)
                                        nc.tensor.matmul(
                                            hsl, rc1[:, cs:cs + P],
                                            wm1[:, h * 512:(h + 1) * 512],
                                            start=False, stop=True)
                                # scale by E[n, d] (broadcast over (br, o))
                                # and reduce over minor d
                                hpt = hpp.tile([P, 2 * DC], F16)
                                nc.vector.tensor_tensor(
                                    out=hpt[:].rearrange(
                                        "p (g d) -> p g d", d=D),
                                    in0=hps[:].rearrange(
                                        "p (g d) -> p g d", d=D),
                                    in1=e_sb[:, t * D:(t + 1) * D]
                                    .unsqueeze(1).broadcast_to((P, 2 * C, D)),
                                    op=OP.mult)
                                nc.vector.tensor_reduce(
                                    out=out_nt[:, bg * P:(bg + 1) * P],
                                    in_=hpt[:].rearrange(
                                        "p (g d) -> p g d", d=D),
                                    axis=mybir.AxisListType.X, op=OP.add)
                            nc.sync.dma_start(
                                out=out_d[:, t * P:(t + 1) * P, :].transpose(
                                    [1, 0, 2]),
                                in_=out_nt[:].rearrange(
                                    "p (b c) -> p b c", c=C))

    nc.compile()
    return nc


_CACHE = {}
_LOCK = threading.Lock()


def _get_program():
    with _LOCK:
        if "nc" not in _CACHE:
            _CACHE["nc"] = build_program()
        return _CACHE["nc"]


def kernel(x, node_embeddings, weights_pool, bias_pool):
    x = np.ascontiguousarray(np.asarray(x, dtype=np.float32))
    emb = np.ascontiguousarray(np.asarray(node_embeddings, dtype=np.float32))
    wp = np.ascontiguousarray(np.asarray(weights_pool, dtype=np.float32))
    bp = np.ascontiguousarray(np.asarray(bias_pool, dtype=np.float32))

    nc = _get_program()
    core_ids = list(range(NCORES))
    in_maps = [
        {"x": x[i * BSH:(i + 1) * BSH], "emb": emb, "wp": wp, "bp": bp}
        for i in core_ids
    ]
    trace = os.environ.get("KERNEL_TRACE", "") == "1"
    res = run_bass_kernel_spmd(nc, in_maps, core_ids, trace=trace)
    if trace:
        kernel.last_exec_time_ns = res.exec_time_ns
        kernel.last_results = res
    out = np.concatenate([res.results[i]["out"] for i in core_ids], axis=0)
    return out


kernel.last_exec_time_ns = None

if __name__ == "__main__":
    rng = np.random.default_rng(0)
    ins = {
        "x": rng.standard_normal((B, N, C), dtype=np.float32),
        "node_embeddings": rng.standard_normal((N, D), dtype=np.float32),
        "weights_pool": (rng.standard_normal((D, K, C, C), dtype=np.float32)
                         * 0.1),
        "bias_pool": rng.standard_normal((D, C), dtype=np.float32) * 0.1,
    }
    out = kernel(**ins)
    print("out", out.shape, out.dtype, float(np.abs(out).mean()))


# revision 8
# speedup vs baseline: 1.7931x; 1.1134x over previous
"""AVWGCN (adaptive-embedding graph conv) Trainium2 Bass kernel.

Math (reference):
    A   = softmax(relu(E E^T), axis=1)            # [N, N], E: [N, D]
    T0  = I, T1 = A, T2 = 2 A A - I               # Chebyshev supports
    W   = einsum('nd,dkio->nkio', E, Wp)          # per-node weights
    b   = E @ bp                                  # per-node bias
    x_g = einsum('knm,bmc->bnkc', T, x)
    out = einsum('bnki,nkio->bno', x_g, W) + b

Restructuring used here (all algebraically exact up to bf16 rounding):
  * Z := exp(relu(E E^T)) = max(exp(E E^T), 1) is SYMMETRIC; with row sums
    s, A = Z/s.  All aggregation matmuls use Z tiles as lhsT directly
    (lhsT.T @ rhs with symmetric Z) and fold 1/s into output scaling.
  * y1 = A @ X,  u2' = 2 A y1  (so y2 = u2' - X^T);  the "- X^T" is folded
    into the weights: W_eff[k0] = Wp[k0] - Wp[k2] applied to x, Wp[k2]
    applied to u2'.
  * Final per-node contraction is d-expanded with (o, d)-major columns:
    H[n,b,(o,d)] = sum_ki R[ki,(b,n)] Wm[ki,(o,d)],
    out[n,b,o] = sum_d E[n,d] H[n,b,(o,d)]
    where R = [x^T; y1^T; u2'^T] ([k*C rows, (b,n) cols]), Wm = Wp_eff.
    The bias is folded in as an extra all-ones contraction row whose weight
    row is bp flattened over (o,d).  The (o,d)-major layout makes the
    E-scale a single stride-0-broadcast multiply and the d-reduction a
    single minor-axis tensor_reduce per PSUM tile.

Sharding: data-parallel over batch B: 8 cores x 8 batches, zero comm.
All matmul operands are bf16 (PSUM accumulation stays fp32).
"""

import os
import sys
import threading

sys.path.insert(0, "/opt/trn_rl_repo")

import numpy as np

import concourse.bass as bass  # noqa: E402
import concourse.mybir as mybir  # noqa: E402
from concourse import bacc  # noqa: E402
from concourse.tile import TileContext  # noqa: E402
from concourse.masks import make_identity  # noqa: E402
from concourse.bass_utils import run_bass_kernel_spmd  # noqa: E402

F32 = mybir.dt.float32
BF = mybir.dt.bfloat16
F16 = mybir.dt.float16
AF = mybir.ActivationFunctionType
OP = mybir.AluOpType

NCORES = 8
B = 64
BSH = B // NCORES          # 8 batches per core
N = 2048
C = 64                     # C_IN == C_OUT
D = 16                     # embedding dim
K = 3                      # Chebyshev order
P = 128                    # partitions
NT = N // P                # 16 node blocks
BC = BSH * C               # 512 = per-core (b, c) width
KI = K * C                 # 192 contraction for the final stage
DC = D * C                 # 1024 (o, d) columns per b in phase E


def build_program():
    nc = bacc.Bacc("TRN2", target_bir_lowering=False, debug=False,
                   num_devices=NCORES)

    x_in = nc.dram_tensor("x", [BSH, N, C], F32, kind="ExternalInput")
    emb = nc.dram_tensor("emb", [N, D], F32, kind="ExternalInput")
    wp = nc.dram_tensor("wp", [D, K, C, C], F32, kind="ExternalInput")
    bp = nc.dram_tensor("bp", [D, C], F32, kind="ExternalInput")
    out_d = nc.dram_tensor("out", [BSH, N, C], F32, kind="ExternalOutput")
    z_dram = nc.dram_tensor("zd", [N, N], BF)  # internal bounce for Z

    with TileContext(nc) as tc:
        with tc.tile_pool(name="persist", bufs=1) as pp:
            ident = pp.tile([P, P], F32)
            make_identity(nc, ident[:])
            identb = pp.tile([P, P], BF)
            make_identity(nc, identb[:])

            # E node-major: [128, (t, d)] and transposed ET [16, 2048] bf16
            e_sb = pp.tile([P, NT * D], F32)
            for t in range(NT):
                nc.sync.dma_start(out=e_sb[:, t * D:(t + 1) * D],
                                  in_=emb[t * P:(t + 1) * P, :])
            et = pp.tile([D, N], BF)

            # Final-stage weights, (o, d)-major columns, bf16.
            wm0 = pp.tile([P, DC], BF)           # rows (k0 c | k1 c)
            wm1 = pp.tile([KI - P + 1, DC], BF)  # rows k2 c + ones-row
            with tc.tile_pool(name="wst", bufs=1) as wst:
                # staging loads keep the DRAM-side layout (d, o) contiguous
                wmst = wst.tile([C, K * DC], F32)
                for k in range(K):
                    nc.sync.dma_start(
                        out=wmst[:, k * DC:(k + 1) * DC].rearrange(
                            "c (d o) -> c d o", o=C),
                        in_=wp[:, k, :, :].transpose([1, 0, 2]))
                bpst = wst.tile([1, DC], F32)
                nc.sync.dma_start(
                    out=bpst[:].rearrange("a (d o) -> a d o", o=C),
                    in_=bp[:, :].unsqueeze(0))

                def odmaj(ap):
                    # view an (o, d)-major destination in (d, o) order
                    return ap.rearrange("c (o d) -> c d o", d=D)

                def domaj(ap):
                    return ap.rearrange("c (d o) -> c d o", o=C)

                # W_eff[k0] = Wp[k0] - Wp[k2]; cast + (o,d)-relayout on write
                nc.vector.tensor_tensor(
                    out=odmaj(wm0[0:C, :]),
                    in0=domaj(wmst[:, 0 * DC:1 * DC]),
                    in1=domaj(wmst[:, 2 * DC:3 * DC]), op=OP.subtract)
                nc.vector.tensor_copy(odmaj(wm0[C:2 * C, :]),
                                      domaj(wmst[:, 1 * DC:2 * DC]))
                nc.vector.tensor_copy(odmaj(wm1[0:C, :]),
                                      domaj(wmst[:, 2 * DC:3 * DC]))
                nc.vector.tensor_copy(
                    wm1[C:C + 1, :].rearrange("c (o d) -> c d o", d=D),
                    bpst[:].rearrange("a (d o) -> a d o", o=C))

            e16 = pp.tile([P, NT * D], F16)
            s_all = pp.tile([P, 2 * NT], F32)
            s_sb = s_all[:, 0:NT]
            sinv = s_all[:, NT:2 * NT]
            sinv2 = pp.tile([P, NT], F32)
            rc0 = pp.tile([P, BSH * N], BF)   # rows: x^T (c) | y1^T (c)

            nc.vector.tensor_copy(e16[:], e_sb[:])

            # ---- build ET via PE transpose (f32 in, cast on copy) ----
            with tc.tile_pool(name="pet", bufs=4, space="PSUM") as pet:
                for t in range(NT):
                    ptile = pet.tile([D, P], F32)
                    nc.tensor.transpose(ptile[:], e_sb[:, t * D:(t + 1) * D],
                                        ident[:])
                    nc.any.tensor_copy(et[:, t * P:(t + 1) * P], ptile[:])

            # ================= Phase B: Z = max(exp(E E^T), 1), s ========
            with tc.tile_pool(name="zb", bufs=2) as zbp, \
                 tc.tile_pool(name="psz", bufs=8, space="PSUM") as psz:
                for t in range(NT):
                    zbt = zbp.tile([P, N], BF)
                    for j in range(4):
                        zt = psz.tile([P, 512], F32)
                        nc.tensor.matmul(zt[:], et[:, t * P:(t + 1) * P],
                                         et[:, j * 512:(j + 1) * 512],
                                         start=True, stop=True)
                        nc.scalar.activation(zbt[:, j * 512:(j + 1) * 512],
                                             zt[:], AF.Exp)
                    # z = max(z, 1);  s[row] = sum(z)
                    nc.vector.tensor_scalar_max(zbt[:], zbt[:], 1.0)
                    nc.vector.tensor_reduce(
                        out=s_sb[:, t:t + 1], in_=zbt[:],
                        axis=mybir.AxisListType.X, op=OP.add)
                    nc.sync.dma_start(out=z_dram[t * P:(t + 1) * P, :],
                                      in_=zbt[:])

            nc.vector.reciprocal(sinv, s_sb)
            nc.vector.tensor_scalar_mul(sinv2[:], sinv, 2.0)

            with tc.tile_pool(name="poolA", bufs=1) as pa:
                y1 = pa.tile([P, NT * BC], BF)
                srep = pa.tile([P, N], F32)   # 2/s[n] replicated on all rows
                srow = pa.tile([1, N], F32)
                with tc.tile_pool(name="pst", bufs=1) as pst, \
                     tc.tile_pool(name="psts", bufs=1, space="PSUM") as psts:
                    stp = psts.tile([D, P], F32)
                    nc.tensor.transpose(stp[:], sinv2[:], ident[:])
                    st_sb = pst.tile([D, P], F32)
                    nc.any.tensor_copy(st_sb[:], stp[:])
                    for t in range(D):
                        nc.sync.dma_start(
                            out=srow[0:1, t * P:(t + 1) * P],
                            in_=st_sb[t:t + 1, :])
                nc.gpsimd.partition_broadcast(srep[:], srow[0:1, :])

                # ============= Phase C: y1 = (Z @ X) / s and x^T =========
                with tc.tile_pool(name="poolB", bufs=1) as pb, \
                     tc.tile_pool(name="xst", bufs=2) as xstp, \
                     tc.tile_pool(name="zc", bufs=2) as zcp, \
                     tc.tile_pool(name="pagg", bufs=2, space="PSUM") as pagg, \
                     tc.tile_pool(name="ptx", bufs=2, space="PSUM") as ptxp:
                    xs = pb.tile([P, NT * BC], BF)
                    for m in range(NT):
                        xst = xstp.tile([P, BC], F32)
                        nc.sync.dma_start(
                            out=xst[:].rearrange("p (b c) -> p b c", c=C),
                            in_=x_in[:, m * P:(m + 1) * P, :].transpose(
                                [1, 0, 2]))
                        nc.scalar.activation(xs[:, m * BC:(m + 1) * BC],
                                             xst[:], AF.Copy)
                    for t in range(NT):
                        zc = zcp.tile([P, N], BF)
                        nc.sync.dma_start(
                            out=zc[:].rearrange("p (mb c) -> p mb c", c=P),
                            in_=z_dram[:, t * P:(t + 1) * P].rearrange(
                                "(mb p) c -> p mb c", p=P))
                        u1 = pagg.tile([P, BC], F32)
                        for m in range(NT):
                            nc.tensor.matmul(
                                u1[:], zc[:, m * P:(m + 1) * P],
                                xs[:, m * BC:(m + 1) * BC],
                                start=(m == 0), stop=(m == NT - 1))
                        nc.vector.tensor_scalar_mul(
                            y1[:, t * BC:(t + 1) * BC], u1[:],
                            sinv[:, t:t + 1])
                    # x^T into Rc0 rows 0..C (k0 slot): batch 8 transposes
                    # per m-block into one PSUM tile, one strided copy out.
                    for m in range(NT):
                        ptx = ptxp.tile([C, BSH * P], BF)
                        for b in range(BSH):
                            nc.tensor.transpose(
                                ptx[:, b * P:(b + 1) * P],
                                xs[:, m * BC + b * C:m * BC + (b + 1) * C],
                                identb[:])
                        nc.any.tensor_copy(
                            rc0[0:C, :].rearrange(
                                "c (b n) -> c b n", b=BSH)[:, :,
                                                           m * P:(m + 1) * P],
                            ptx[:].rearrange("c (b n) -> c b n", b=BSH))

                # ============= Phase D: y1^T, u2' = 2 (Z @ y1) / s =======
                # y1^T transposes target PSUM partitions 64..127 directly
                # (tile_position col offset 64), then a lane-locked copy
                # lands them in rc0 rows 64..127.
                with tc.tile_pool(name="pty", bufs=2, space="PSUM") as ptyp:
                    for m in range(NT):
                        pty = ptyp.tile([P, BSH * P], BF)
                        for b in range(BSH):
                            nc.tensor.transpose(
                                pty[C:P, b * P:(b + 1) * P],
                                y1[:, m * BC + b * C:m * BC + (b + 1) * C],
                                identb[:],
                                tile_position=(0, C))
                        nc.any.tensor_copy(
                            rc0[C:P, :].rearrange(
                                "c (b n) -> c b n", b=BSH)[:, :,
                                                           m * P:(m + 1) * P],
                            pty[C:P, :].rearrange("c (b n) -> c b n", b=BSH))

                with tc.tile_pool(name="poolC", bufs=1) as pcp:
                    rc1 = pcp.tile([KI - P + 1, BSH * N], BF)
                    nc.vector.memset(rc1[C:C + 1, :], 1.0)
                    with tc.tile_pool(name="zl", bufs=3) as zlp, \
                         tc.tile_pool(name="pu2", bufs=2,
                                      space="PSUM") as pu2, \
                         tc.tile_pool(name="y2p", bufs=2) as y2p:
                        for nq in range(4):
                            u2t = pu2.tile([P, 4 * 512], F32)
                            for m in range(NT):
                                zl = zlp.tile([P, 512], BF)
                                nc.sync.dma_start(
                                    out=zl[:],
                                    in_=z_dram[m * P:(m + 1) * P,
                                               nq * 512:(nq + 1) * 512])
                                for bc in range(4):
                                    nc.tensor.matmul(
                                        u2t[:, bc * 512:(bc + 1) * 512],
                                        y1[:, m * BC + bc * P:
                                           m * BC + (bc + 1) * P],
                                        zl[:],
                                        start=(m == 0), stop=(m == NT - 1))
                            # y2 = u2t * (2/s[n]) in one broadcast multiply
                            y2t = y2p.tile([P, 4 * 512], BF)
                            nc.vector.tensor_tensor(
                                out=y2t[:].rearrange(
                                    "p (q n) -> p q n", q=4),
                                in0=u2t[:].rearrange(
                                    "p (q n) -> p q n", q=4),
                                in1=srep[:, nq * 512:(nq + 1) * 512]
                                .unsqueeze(1).broadcast_to((P, 4, 512)),
                                op=OP.mult)
                            for bc in range(4):
                                for h in range(2):
                                    bb = 2 * bc + h
                                    dst = rc1[0:C, bb * N + nq * 512:
                                              bb * N + (nq + 1) * 512]
                                    src = y2t[h * C:(h + 1) * C,
                                              bc * 512:(bc + 1) * 512]
                                    if h == 0:
                                        nc.scalar.activation(dst, src,
                                                             AF.Copy)
                                    else:
                                        nc.sync.dma_start(out=dst, in_=src)

                    # ============= Phase E: final ========================
                    # Pipeline per (t, bg): PE matmuls -> ACT cast to fp16
                    # -> DVE/Pool broadcast E-multiply -> DVE d-reduce
                    # (fp16 accum over 16 terms) -> ACT cast to f32 -> DMA.
                    with tc.tile_pool(name="psH", bufs=2,
                                      space="PSUM") as psh, \
                         tc.tile_pool(name="hs", bufs=3) as hsp, \
                         tc.tile_pool(name="hp", bufs=3) as hpp, \
                         tc.tile_pool(name="outp", bufs=2) as outp, \
                         tc.tile_pool(name="outf", bufs=2) as outfp, \
                         nc.allow_low_precision(
                             reason="d-reduce over 16 fp16 terms"):
                        for t in range(NT):
                            out_nt = outp.tile([P, BC], F16)
                            for bg in range(4):
                                hps = psh.tile([P, 2 * DC], F32)
                                for br in range(2):
                                    b = bg * 2 + br
                                    cs = b * N + t * P
                                    for h in range(2):
                                        hsl = hps[:, br * DC + h * 512:
                                                  br * DC + (h + 1) * 512]
                                        nc.tensor.matmul(
                                            hsl, rc0[:, cs:cs + P],
                                            wm0[:, h * 512:(h + 1) * 512],
                                            start=True, stop=False)
                                        nc.tensor.matmul(
                                            hsl, rc1[:, cs:cs + P],
                                            wm1[:, h * 512:(h + 1) * 512],
                                            start=False, stop=True)
                                hsb = hsp.tile([P, 2 * DC], F16)
                                nc.scalar.activation(hsb[:], hps[:], AF.Copy)
                                # scale by E[n, d] (broadcast over (br, o))
                                # and reduce over minor d
                                hpt = hpp.tile([P, 2 * DC], F16)
                                eng = nc.vector if bg % 2 else nc.gpsimd
                                eng.tensor_tensor(
                                    out=hpt[:].rearrange(
                                        "p (g d) -> p g d", d=D),
                                    in0=hsb[:].rearrange(
                                        "p (g d) -> p g d", d=D),
                                    in1=e16[:, t * D:(t + 1) * D]
                                    .unsqueeze(1).broadcast_to((P, 2 * C, D)),
                                    op=OP.mult)
                                nc.vector.tensor_reduce(
                                    out=out_nt[:, bg * P:(bg + 1) * P],
                                    in_=hpt[:].rearrange(
                                        "p (g d) -> p g d", d=D),
                                    axis=mybir.AxisListType.X, op=OP.add)
                            outf = outfp.tile([P, BC], F32)
                            nc.scalar.activation(outf[:], out_nt[:], AF.Copy)
                            nc.sync.dma_start(
                                out=out_d[:, t * P:(t + 1) * P, :].transpose(
                                    [1, 0, 2]),
                                in_=outf[:].rearrange(
                                    "p (b c) -> p b c", c=C))

    nc.compile()
    return nc


_CACHE = {}
_LOCK = threading.Lock()


def _get_program():
    with _LOCK:
        if "nc" not in _CACHE:
            _CACHE["nc"] = build_program()
        return _CACHE["nc"]


def kernel(x, node_embeddings, weights_pool, bias_pool):
    x = np.ascontiguousarray(np.asarray(x, dtype=np.float32))
    emb = np.ascontiguousarray(np.asarray(node_embeddings, dtype=np.float32))
    wp = np.ascontiguousarray(np.asarray(weights_pool, dtype=np.float32))
    bp = np.ascontiguousarray(np.asarray(bias_pool, dtype=np.float32))

    nc = _get_program()
    core_ids = list(range(NCORES))
    in_maps = [
        {"x": x[i * BSH:(i + 1) * BSH], "emb": emb, "wp": wp, "bp": bp}
        for i in core_ids
    ]
    trace = os.environ.get("KERNEL_TRACE", "") == "1"
    res = run_bass_kernel_spmd(nc, in_maps, core_ids, trace=trace)
    if trace:
        kernel.last_exec_time_ns = res.exec_time_ns
        kernel.last_results = res
    out = np.concatenate([res.results[i]["out"] for i in core_ids], axis=0)
    return out


kernel.last_exec_time_ns = None

if __name__ == "__main__":
    rng = np.random.default_rng(0)
    ins = {
        "x": rng.standard_normal((B, N, C), dtype=np.float32),
        "node_embeddings": rng.standard_normal((N, D), dtype=np.float32),
        "weights_pool": (rng.standard_normal((D, K, C, C), dtype=np.float32)
                         * 0.1),
        "bias_pool": rng.standard_normal((D, C), dtype=np.float32) * 0.1,
    }
    out = kernel(**ins)
    print("out", out.shape, out.dtype, float(np.abs(out).mean()))


# revision 22
# speedup vs baseline: 1.8997x; 1.0594x over previous
"""AVWGCN (adaptive-embedding graph conv) Trainium2 Bass kernel.

Math (reference):
    A   = softmax(relu(E E^T), axis=1)            # [N, N], E: [N, D]
    T0  = I, T1 = A, T2 = 2 A A - I               # Chebyshev supports
    W   = einsum('nd,dkio->nkio', E, Wp)          # per-node weights
    b   = E @ bp                                  # per-node bias
    x_g = einsum('knm,bmc->bnkc', T, x)
    out = einsum('bnki,nkio->bno', x_g, W) + b

Restructuring used here (all algebraically exact up to bf16 rounding):
  * Z := exp(relu(E E^T)) = max(exp(E E^T), 1) is SYMMETRIC; with row sums
    s, A = Z/s.  All aggregation matmuls use Z tiles as lhsT directly
    (lhsT.T @ rhs with symmetric Z) and fold 1/s into output scaling.
  * y1 = A @ X,  u2' = 2 A y1  (so y2 = u2' - X^T);  the "- X^T" is folded
    into the weights: W_eff[k0] = Wp[k0] - Wp[k2] applied to x, Wp[k2]
    applied to u2'.
  * Final per-node contraction is d-expanded with (o, d)-major columns:
    H[n,b,(o,d)] = sum_ki R[ki,(b,n)] Wm[ki,(o,d)],
    out[n,b,o] = sum_d E[n,d] H[n,b,(o,d)]
    where R = [x^T; y1^T; u2'^T] ([k*C rows, (b,n) cols]), Wm = Wp_eff.
    The bias is folded in as an extra all-ones contraction row whose weight
    row is bp flattened over (o,d).  The (o,d)-major layout makes the
    E-scale a single stride-0-broadcast multiply and the d-reduction a
    single minor-axis tensor_reduce per PSUM tile.

Sharding: data-parallel over batch B: 8 cores x 8 batches, zero comm.
All matmul operands are bf16 (PSUM accumulation stays fp32).
"""

import os
import sys
import threading

sys.path.insert(0, "/opt/trn_rl_repo")

import numpy as np

import concourse.bass as bass  # noqa: E402
import concourse.mybir as mybir  # noqa: E402
from concourse import bacc  # noqa: E402
from concourse.tile import TileContext  # noqa: E402
from concourse.masks import make_identity  # noqa: E402
from concourse.bass_utils import run_bass_kernel_spmd  # noqa: E402

F32 = mybir.dt.float32
BF = mybir.dt.bfloat16
F16 = mybir.dt.float16
AF = mybir.ActivationFunctionType
OP = mybir.AluOpType

NCORES = 8
B = 64
BSH = B // NCORES          # 8 batches per core
N = 2048
C = 64                     # C_IN == C_OUT
D = 16                     # embedding dim
K = 3                      # Chebyshev order
P = 128                    # partitions
NT = N // P                # 16 node blocks
BC = BSH * C               # 512 = per-core (b, c) width
KI = K * C                 # 192 contraction for the final stage
DC = D * C                 # 1024
NSH = N // NCORES          # 256 nodes per core in the final stage
KR = KI - P + 1            # 65: rows of the second R tile (y2 + ones)


def build_program():
    nc = bacc.Bacc("TRN2", target_bir_lowering=False, debug=False,
                   num_devices=NCORES)

    x_in = nc.dram_tensor("x", [BSH, N, C], F32, kind="ExternalInput")
    emb = nc.dram_tensor("emb", [N, D], F32, kind="ExternalInput")
    wp = nc.dram_tensor("wp", [D, K, C, C], F32, kind="ExternalInput")
    bp = nc.dram_tensor("bp", [D, C], F32, kind="ExternalInput")
    # node-sharded output: this core's NSH nodes, all B batches
    out_d = nc.dram_tensor("out", [B, NSH, C], F32, kind="ExternalOutput")
    z_dram = nc.dram_tensor("zd", [N, N], BF)  # internal bounce for Z
    GROUP = [list(range(NCORES))]

    with TileContext(nc) as tc:
        with tc.tile_pool(name="persist", bufs=1) as pp, \
             tc.tile_pool(name="ccd", bufs=1, space="DRAM") as ccd:
            ident = pp.tile([P, P], F32)
            make_identity(nc, ident[:])
            identb = pp.tile([P, P], BF)
            make_identity(nc, identb[:])

            # E node-major: [128, (t, d)] and transposed ET [16, 2048] bf16
            e_sb = pp.tile([P, NT * D], F32)
            for t in range(NT):
                nc.sync.dma_start(out=e_sb[:, t * D:(t + 1) * D],
                                  in_=emb[t * P:(t + 1) * P, :])
            et = pp.tile([D, N], BF)

            # Weight-pool staging for the per-node final stage.
            # wk01[d, o*128 + k*64 + i] = Wp_eff[d, k, i, o]  (k in {0,1})
            # wk2 [d, o*65 + i]         = Wp[d, 2, i, o];  i=64 col = bp[d, o]
            wk01 = pp.tile([D, P * C], BF)
            wk2 = pp.tile([D, (C + 1) * C], BF)
            with tc.tile_pool(name="wst", bufs=1) as wst:
                wpst = wst.tile([D, K * C * C], F32)
                for k in range(K):
                    nc.sync.dma_start(
                        out=wpst[:, k * C * C:(k + 1) * C * C],
                        in_=wp[:, k, :, :].rearrange("d i o -> d (i o)"))
                bpst = wst.tile([D, C], F32)
                nc.sync.dma_start(out=bpst[:], in_=bp[:, :])

                def kslice(k):
                    # view wpst k-slice as [d, o, i] (strided read)
                    return wpst[:, k * C * C:(k + 1) * C * C].rearrange(
                        "d (i o) -> d o i", o=C)

                def w01view(k):
                    return wk01[:].rearrange(
                        "d (o ki) -> d o ki", ki=P)[:, :, k * C:(k + 1) * C]

                # W_eff[k0] = Wp[k0] - Wp[k2] (folds the -x of T2 = 2AA - I)
                nc.vector.tensor_tensor(out=w01view(0), in0=kslice(0),
                                        in1=kslice(2), op=OP.subtract)
                nc.vector.tensor_copy(w01view(1), kslice(1))
                nc.vector.tensor_copy(
                    wk2[:].rearrange("d (o i) -> d o i", i=C + 1)[:, :, 0:C],
                    kslice(2))
                nc.vector.tensor_copy(
                    wk2[:].rearrange("d (o i) -> d o i", i=C + 1)[:, :,
                                                                 C:C + 1],
                    bpst[:].unsqueeze(2))

            s_all = pp.tile([P, 2 * NT], F32)
            s_sb = s_all[:, 0:NT]
            sinv = s_all[:, NT:2 * NT]
            sinv2 = pp.tile([P, NT], F32)
            et_r = pp.tile([D, NSH], BF)  # E^T slice for this core's nodes

            # ---- build ET via PE transpose (f32 in, cast on copy) ----
            with tc.tile_pool(name="pet", bufs=4, space="PSUM") as pet:
                for t in range(NT):
                    ptile = pet.tile([D, P], F32)
                    nc.tensor.transpose(ptile[:], e_sb[:, t * D:(t + 1) * D],
                                        ident[:])
                    nc.any.tensor_copy(et[:, t * P:(t + 1) * P], ptile[:])

            # DRAM bounce tensors for the collectives.
            esend = ccd.tile([NCORES, D, NSH], BF)
            erecv = ccd.tile([NCORES, D, NSH], BF)
            r0send = ccd.tile([NCORES, P, BSH * NSH], BF)
            r0recv = ccd.tile([NCORES, P, BSH * NSH], BF)
            r1send = ccd.tile([NCORES, KR, BSH * NSH], BF)
            r1recv = ccd.tile([NCORES, KR, BSH * NSH], BF)

            # Tiny AllToAll: every core sends chunk j = E^T for core j's
            # nodes; every recv chunk holds MY node range (use chunk 0).
            for j in range(NCORES):
                nc.sync.dma_start(out=esend[j, :, :],
                                  in_=et[:, j * NSH:(j + 1) * NSH])
            nc.gpsimd.collective_compute(
                "AllToAll", OP.bypass, replica_groups=GROUP,
                ins=[esend.opt()], outs=[erecv.opt()])
            nc.sync.dma_start(out=et_r[:], in_=erecv[0, :, :])

            # ================= Phase B: Z = max(exp(E E^T), 1), s ========
            with tc.tile_pool(name="zb", bufs=2) as zbp, \
                 tc.tile_pool(name="psz", bufs=8, space="PSUM") as psz:
                for t in range(NT):
                    zbt = zbp.tile([P, N], BF)
                    for j in range(4):
                        zt = psz.tile([P, 512], F32)
                        nc.tensor.matmul(zt[:], et[:, t * P:(t + 1) * P],
                                         et[:, j * 512:(j + 1) * 512],
                                         start=True, stop=True)
                        nc.scalar.activation(zbt[:, j * 512:(j + 1) * 512],
                                             zt[:], AF.Exp)
                    # z = max(z, 1);  s[row] = sum(z)
                    nc.vector.tensor_scalar_max(zbt[:], zbt[:], 1.0)
                    nc.vector.tensor_reduce(
                        out=s_sb[:, t:t + 1], in_=zbt[:],
                        axis=mybir.AxisListType.X, op=OP.add)
                    nc.sync.dma_start(out=z_dram[t * P:(t + 1) * P, :],
                                      in_=zbt[:])

            nc.vector.reciprocal(sinv, s_sb)
            nc.vector.tensor_scalar_mul(sinv2[:], sinv, 2.0)

            with tc.tile_pool(name="rcp", bufs=1) as rcp, \
                 tc.tile_pool(name="poolA", bufs=1) as pa:
                rc0 = rcp.tile([P, BSH * N], BF)  # rows: x^T | y1^T
                y1 = pa.tile([P, NT * BC], BF)
                srep = pa.tile([P, N], F32)   # 2/s[n] replicated on all rows
                srow = pa.tile([1, N], F32)
                with tc.tile_pool(name="pst", bufs=1) as pst, \
                     tc.tile_pool(name="psts", bufs=1, space="PSUM") as psts:
                    stp = psts.tile([D, P], F32)
                    nc.tensor.transpose(stp[:], sinv2[:], ident[:])
                    st_sb = pst.tile([D, P], F32)
                    nc.any.tensor_copy(st_sb[:], stp[:])
                    for t in range(D):
                        nc.sync.dma_start(
                            out=srow[0:1, t * P:(t + 1) * P],
                            in_=st_sb[t:t + 1, :])
                nc.gpsimd.partition_broadcast(srep[:], srow[0:1, :])

                # ============= Phase C: y1 = (Z @ X) / s and x^T =========
                with tc.tile_pool(name="poolB", bufs=1) as pb, \
                     tc.tile_pool(name="xst", bufs=2) as xstp, \
                     tc.tile_pool(name="zc", bufs=2) as zcp, \
                     tc.tile_pool(name="pagg", bufs=2, space="PSUM") as pagg, \
                     tc.tile_pool(name="ptx", bufs=2, space="PSUM") as ptxp:
                    xs = pb.tile([P, NT * BC], BF)
                    for m in range(NT):
                        xst = xstp.tile([P, BC], F32)
                        nc.sync.dma_start(
                            out=xst[:].rearrange("p (b c) -> p b c", c=C),
                            in_=x_in[:, m * P:(m + 1) * P, :].transpose(
                                [1, 0, 2]))
                        nc.scalar.activation(xs[:, m * BC:(m + 1) * BC],
                                             xst[:], AF.Copy)
                    for t in range(NT):
                        zc = zcp.tile([P, N], BF)
                        nc.sync.dma_start(
                            out=zc[:].rearrange("p (mb c) -> p mb c", c=P),
                            in_=z_dram[:, t * P:(t + 1) * P].rearrange(
                                "(mb p) c -> p mb c", p=P))
                        u1 = pagg.tile([P, BC], F32)
                        for m in range(NT):
                            nc.tensor.matmul(
                                u1[:], zc[:, m * P:(m + 1) * P],
                                xs[:, m * BC:(m + 1) * BC],
                                start=(m == 0), stop=(m == NT - 1))
                        nc.vector.tensor_scalar_mul(
                            y1[:, t * BC:(t + 1) * BC], u1[:],
                            sinv[:, t:t + 1])
                    # x^T into Rc0 rows 0..C (k0 slot): batch 8 transposes
                    # per m-block into one PSUM tile, one strided copy out.
                    for m in range(NT):
                        ptx = ptxp.tile([C, BSH * P], BF)
                        for b in range(BSH):
                            nc.tensor.transpose(
                                ptx[:, b * P:(b + 1) * P],
                                xs[:, m * BC + b * C:m * BC + (b + 1) * C],
                                identb[:])
                        nc.any.tensor_copy(
                            rc0[0:C, :].rearrange(
                                "c (b n) -> c b n", b=BSH)[:, :,
                                                           m * P:(m + 1) * P],
                            ptx[:].rearrange("c (b n) -> c b n", b=BSH))

                # ============= Phase D: y1^T, u2' = 2 (Z @ y1) / s =======
                # y1^T transposes target PSUM partitions 64..127 directly
                # (tile_position col offset 64), then a lane-locked copy
                # lands them in rc0 rows 64..127.
                with tc.tile_pool(name="pty", bufs=2, space="PSUM") as ptyp:
                    for m in range(NT):
                        pty = ptyp.tile([P, BSH * P], BF)
                        for b in range(BSH):
                            nc.tensor.transpose(
                                pty[C:P, b * P:(b + 1) * P],
                                y1[:, m * BC + b * C:m * BC + (b + 1) * C],
                                identb[:],
                                tile_position=(0, C))
                        nc.any.tensor_copy(
                            rc0[C:P, :].rearrange(
                                "c (b n) -> c b n", b=BSH)[:, :,
                                                           m * P:(m + 1) * P],
                            pty[C:P, :].rearrange("c (b n) -> c b n", b=BSH))

                # rc0 ready: exchange it while Phase D2 computes u2'.
                for j in range(NCORES):
                    nc.sync.dma_start(
                        out=r0send[j, :, :].rearrange(
                            "p (b n) -> p b n", n=NSH),
                        in_=rc0[:, :].rearrange(
                            "p (b n) -> p b n", n=N)[:, :,
                                                     j * NSH:(j + 1) * NSH])
                nc.gpsimd.collective_compute(
                    "AllToAll", OP.bypass, replica_groups=GROUP,
                    ins=[r0send.opt()], outs=[r0recv.opt()])

                with tc.tile_pool(name="poolC", bufs=1) as pcp:
                    rc1 = pcp.tile([KI - P + 1, BSH * N], BF)
                    nc.vector.memset(rc1[C:C + 1, :], 1.0)
                    with tc.tile_pool(name="zl", bufs=3) as zlp, \
                         tc.tile_pool(name="pu2", bufs=2,
                                      space="PSUM") as pu2, \
                         tc.tile_pool(name="y2p", bufs=2) as y2p:
                        for nq in range(4):
                            u2t = pu2.tile([P, 4 * 512], F32)
                            for m in range(NT):
                                zl = zlp.tile([P, 512], BF)
                                nc.sync.dma_start(
                                    out=zl[:],
                                    in_=z_dram[m * P:(m + 1) * P,
                                               nq * 512:(nq + 1) * 512])
                                for bc in range(4):
                                    nc.tensor.matmul(
                                        u2t[:, bc * 512:(bc + 1) * 512],
                                        y1[:, m * BC + bc * P:
                                           m * BC + (bc + 1) * P],
                                        zl[:],
                                        start=(m == 0), stop=(m == NT - 1))
                            # y2 = u2t * (2/s[n]) in one broadcast multiply
                            y2t = y2p.tile([P, 4 * 512], BF)
                            nc.vector.tensor_tensor(
                                out=y2t[:].rearrange(
                                    "p (q n) -> p q n", q=4),
                                in0=u2t[:].rearrange(
                                    "p (q n) -> p q n", q=4),
                                in1=srep[:, nq * 512:(nq + 1) * 512]
                                .unsqueeze(1).broadcast_to((P, 4, 512)),
                                op=OP.mult)
                            for bc in range(4):
                                for h in range(2):
                                    bb = 2 * bc + h
                                    dst = rc1[0:C, bb * N + nq * 512:
                                              bb * N + (nq + 1) * 512]
                                    src = y2t[h * C:(h + 1) * C,
                                              bc * 512:(bc + 1) * 512]
                                    if h == 0:
                                        nc.scalar.activation(dst, src,
                                                             AF.Copy)
                                    else:
                                        nc.sync.dma_start(out=dst, in_=src)

                    # rc1 ready: exchange it.
                    for j in range(NCORES):
                        nc.sync.dma_start(
                            out=r1send[j, :, :].rearrange(
                                "p (b n) -> p b n", n=NSH),
                            in_=rc1[:, :].rearrange(
                                "p (b n) -> p b n",
                                n=N)[:, :, j * NSH:(j + 1) * NSH])
                    nc.gpsimd.collective_compute(
                        "AllToAll", OP.bypass, replica_groups=GROUP,
                        ins=[r1send.opt()], outs=[r1recv.opt()])

            # ============= Phase F: per-node final stage =================
            # Rr0/Rr1: R columns for ALL 64 batches x my NSH nodes,
            # cols = (b_global, n_local) = (src*8 + b_local, n).
            # W0[ki, o*NSH + n] = W_eff[n, ki, o] (ki = k0 i | k1 i)
            # W1[i,  o*NSH + n] = W[n, k2, i, o]; row 64 = bias[n, o].
            # out[b, n, o] for node n: 2 accumulating matmuls with
            # lhsT = Rr[:, b-cols of n] (stride NSH), rhs = W[:, o-cols of n]
            # (stride NSH), packed 2 nodes x 8 nodes per PSUM bank.
            with tc.tile_pool(name="poolF", bufs=1) as pf:
                rr0 = pf.tile([P, B * NSH], BF)
                rr1 = pf.tile([KR, B * NSH], BF)
                w0 = pf.tile([P, C * NSH], BF)
                w1 = pf.tile([KR, C * NSH], BF)
                for i in range(NCORES):
                    nc.sync.dma_start(
                        out=rr0[:, i * BSH * NSH:(i + 1) * BSH * NSH],
                        in_=r0recv[i, :, :])
                    nc.sync.dma_start(
                        out=rr1[:, i * BSH * NSH:(i + 1) * BSH * NSH],
                        in_=r1recv[i, :, :])

                # W build: per output channel o, W[:, o-block] = wk^T @ E_r
                with tc.tile_pool(name="psW", bufs=4, space="PSUM") as psw, \
                     tc.tile_pool(name="psV", bufs=4, space="PSUM") as psv:
                    for op_ in range(C // 2):
                        pw = psw.tile([P, 2 * NSH], F32)
                        pv = psv.tile([KR, 2 * NSH], F32)
                        for h in range(2):
                            o = 2 * op_ + h
                            nc.tensor.matmul(
                                pw[:, h * NSH:(h + 1) * NSH],
                                wk01[:, o * P:(o + 1) * P],
                                et_r[:], start=True, stop=True)
                            nc.tensor.matmul(
                                pv[:, h * NSH:(h + 1) * NSH],
                                wk2[:, o * (C + 1):(o + 1) * (C + 1)],
                                et_r[:], start=True, stop=True)
                        nc.any.tensor_copy(
                            w0[:, 2 * op_ * NSH:(2 * op_ + 2) * NSH], pw[:])
                        nc.any.tensor_copy(
                            w1[:, 2 * op_ * NSH:(2 * op_ + 2) * NSH], pv[:])

                rr0v = rr0[:, :].rearrange("p (b n) -> p b n", n=NSH)
                rr1v = rr1[:, :].rearrange("p (b n) -> p b n", n=NSH)
                w0v = w0[:, :].rearrange("p (o n) -> p o n", n=NSH)
                w1v = w1[:, :].rearrange("p (o n) -> p o n", n=NSH)
                NG = NSH // 16  # node groups of 16 (2 part-halves x 8 slots)
                with tc.tile_pool(name="psF", bufs=4, space="PSUM") as psf, \
                     tc.tile_pool(name="outp", bufs=2) as outp:
                    for g in range(NG):
                        pout = psf.tile([P, 512], F32)
                        for idx in range(16):
                            n = 16 * g + idx
                            s, j = idx % 2, idx // 2
                            osl = pout[C * s:C * (s + 1),
                                       j * C:(j + 1) * C]
                            nc.tensor.matmul(
                                osl, rr0v[:, :, n], w0v[:, :, n],
                                start=True, stop=False,
                                tile_position=(0, C * s))
                            nc.tensor.matmul(
                                osl, rr1v[:, :, n], w1v[:, :, n],
                                start=False, stop=True,
                                tile_position=(0, C * s))
                        outsb = outp.tile([P, 512], F32)
                        nc.any.tensor_copy(outsb[:], pout[:])
                        for s in range(2):
                            nc.sync.dma_start(
                                out=out_d[:, g * 16:(g + 1) * 16, :]
                                .rearrange("b (j s) c -> s b j c",
                                           s=2)[s:s + 1],
                                in_=outsb[C * s:C * (s + 1), :].rearrange(
                                    "b (j c) -> b j c", c=C))

    nc.compile()
    return nc


_CACHE = {}
_LOCK = threading.Lock()


def _get_program():
    with _LOCK:
        if "nc" not in _CACHE:
            _CACHE["nc"] = build_program()
        return _CACHE["nc"]


def kernel(x, node_embeddings, weights_pool, bias_pool):
    x = np.ascontiguousarray(np.asarray(x, dtype=np.float32))
    emb = np.ascontiguousarray(np.asarray(node_embeddings, dtype=np.float32))
    wp = np.ascontiguousarray(np.asarray(weights_pool, dtype=np.float32))
    bp = np.ascontiguousarray(np.asarray(bias_pool, dtype=np.float32))

    nc = _get_program()
    core_ids = list(range(NCORES))
    in_maps = [
        {"x": x[i * BSH:(i + 1) * BSH], "emb": emb, "wp": wp, "bp": bp}
        for i in core_ids
    ]
    trace = os.environ.get("KERNEL_TRACE", "") == "1"
    res = run_bass_kernel_spmd(nc, in_maps, core_ids, trace=trace)
    if trace:
        kernel.last_exec_time_ns = res.exec_time_ns
        kernel.last_results = res
    out = np.concatenate([res.results[i]["out"] for i in core_ids], axis=1)
    return out


kernel.last_exec_time_ns = None

if __name__ == "__main__":
    rng = np.random.default_rng(0)
    ins = {
        "x": rng.standard_normal((B, N, C), dtype=np.float32),
        "node_embeddings": rng.standard_normal((N, D), dtype=np.float32),
        "weights_pool": (rng.standard_normal((D, K, C, C), dtype=np.float32)
                         * 0.1),
        "bias_pool": rng.standard_normal((D, C), dtype=np.float32) * 0.1,
    }
    out = kernel(**ins)
    print("out", out.shape, out.dtype, float(np.abs(out).mean()))
